# revision 8
# baseline (speedup 1.0000x reference)
"""Self-contained 8-core Trainium2 (Bass/Tile) kernel for a 2-layer GCN
(DGI-style) + global mean pool + linear summary head.

Reference computation (PyG GCNConv defaults):
    h1 = relu(Dh(x W1) + b1);  h = Dh(h1 W2) + b2
    where Dh y = dinv * (A^T (dinv*y) + dinv*y),  deg = indeg + 1
    g = segment_mean(h, batch);  summary = g Ws + bs
    returns (summary, h)

Sharding: nodes in 8 contiguous blocks (graph/node partition); edges
partitioned by dst owner; weights replicated; full hs tables exchanged
via AllGather between layers.

Device algorithm per core ("transposed land" - features on partitions):
  - h1T = W1^T xT on PE (xT host-pretransposed); hsT = dinv * hT.
  - un-transpose hsT tiles via PE; DMA rows into a 256B-strided
    AllGather bounce; AllGather -> row-major gather table in DRAM.
  - aggregation: per-core edges sorted by dst are packed into 128-slot
    chunks, each chunk pure in (64-node dst band, 25600-row src window);
    int16-indexed dma_gather (8-Q7 ucode) fetches 128B message rows per
    window segment; one-hot selection matrices built on DVE; PE matmuls
    (msg^T @ S) merge chunks into per-band PSUM columns.
  - epilogue: out = dinv*(agg + hs_own) + b (+relu for layer 1).
  - pooling: one-hot graph matmuls -> gsum/cnt PSUM; AllReduce;
    summary = (gsum/max(cnt,1)) Ws + bs.
"""
import math

import numpy as np

import concourse.bass as bass
import concourse.bacc as bacc
import concourse.tile as tile
from concourse import bass_utils, mybir

F32 = mybir.dt.float32
I16 = mybir.dt.int16

# ---- problem constants (hardcoded per contract) ----
N_NODES = 100000
N_EDGES = 1600000
IN_CH = 128
HID = 32
N_GRAPHS = 64
N_CORES = 8

BP = 6          # tile-pairs per aggregation batch
MAX_SEG = 56    # max chunks per dma_gather call (ring limit safety)


def _ceil_to(a, m):
    return (a + m - 1) // m * m


def dma_gather_rows(eng, out_ap, in_ap, idxs_ap, num_idxs, elem_size,
                    elem_step, queue_num=0):
    """bass.dma_gather (non-transpose, DRAM source) allowing payloads that
    are not 256B multiples; the row stride (elem_step) must be."""
    stride_bytes = elem_step * mybir.dt.size(in_ap.dtype)
    assert stride_bytes % 256 == 0 and stride_bytes // 256 < 256
    assert idxs_ap.dtype == I16
    assert in_ap.dtype == out_ap.dtype
    assert in_ap.ap[0][0] == elem_step and in_ap.ap[-1][1] == elem_size
    _in_ap = eng.lower_ap_dma(in_ap, for_custom_bir_dma=True)
    _idxs_ap = eng.lower_ap(idxs_ap)
    _out_ap = eng.lower_ap(out_ap)
    return eng.add_instruction(
        mybir.InstDMAGatherAnt(
            name=eng.bass.get_next_instruction_name(),
            ins=[*_in_ap, _idxs_ap,
                 eng.lower_val_access(eng.to_reg(num_idxs))],
            outs=[_out_ap],
            transpose=False,
            num_idxs=num_idxs,
            elem_size=elem_size,
            stride_bytes_256=stride_bytes // 256,
            gen_mode=0,
            single_packet=False,
            queue_num=queue_num,
            sbuf_tokens_per_rank=0,
            sbuf_free_dim_per_rank=0,
            sbuf_free_dim_pad_per_rank=0,
            sbuf_byte_offset=0,
        ))


# ======================================================================
# Host-side preprocessing: shard + sort + pad (index-only manipulation)
# ======================================================================

def _plan_chunks(cfg, counts):
    """Uniform chunk directory from per-core (band, window) counts.

    counts: [nc][nb][W] -> edges. Returns K_bw [nb][W], directory list of
    (band, window, k), per-window chunk counts NW, stream pos of each
    chunk, and batch structure."""
    nb, W = cfg["nb"], cfg["W"]
    K = np.zeros((nb, W), np.int64)
    for c in range(cfg["nc"]):
        K = np.maximum(K, -(-counts[c] // 128))
    for b in range(nb):
        if K[b].sum() == 0:
            K[b][0] = 1           # every band needs one (start=True) chunk
    directory = []
    pw = [0] * W
    stream_pos = {}
    for b in range(nb):
        for w in range(W):
            for k in range(int(K[b][w])):
                stream_pos[(b, w, k)] = pw[w]
                directory.append((b, w, k))
                pw[w] += 1
    NW = list(pw)
    return K, directory, NW, stream_pos


def _prep(x, edge_index, batch, W1, b1, W2, b2, Ws, bs):
    x = np.asarray(x, np.float32)
    edge_index = np.asarray(edge_index, np.int32)
    batch = np.asarray(batch, np.int32)
    W1 = np.asarray(W1, np.float32)
    b1 = np.asarray(b1, np.float32)
    W2 = np.asarray(W2, np.float32)
    b2 = np.asarray(b2, np.float32)
    Ws = np.asarray(Ws, np.float32)
    bs = np.asarray(bs, np.float32)

    n, in_ch = x.shape
    hid = W1.shape[1]
    g = N_GRAPHS
    nc_ = N_CORES
    assert n % nc_ == 0
    per = n // nc_
    npad = _ceil_to(per, 256)
    T = npad // 128
    nb = npad // 64                   # 64-node bands
    W = nc_ // 2                      # source windows (rank pairs)
    WROWS = 2 * npad
    assert WROWS <= 32767

    src = edge_index[0].astype(np.int64)
    dst = edge_index[1].astype(np.int64)
    deg = np.bincount(dst, minlength=n).astype(np.float32) + 1.0

    order = np.argsort(dst, kind="stable")
    dsts = dst[order]
    srcs = src[order]
    core_starts = np.searchsorted(dsts, np.arange(nc_ + 1) * per)

    cfg = dict(n=n, per=per, npad=npad, T=T, nb=nb, W=W, WROWS=WROWS,
               in_ch=in_ch, hid=hid, g=g, nc=nc_)

    # table row + window of each source node
    def trow(v):
        return (v // per) * npad + (v % per)

    # per-core per-(band, window) edge lists
    counts = []
    core_edges = []
    for c in range(nc_):
        lo, hi = core_starts[c], core_starts[c + 1]
        d_loc = (dsts[lo:hi] - c * per).astype(np.int64)
        s_glob = srcs[lo:hi]
        rows = (s_glob // per) * npad + (s_glob % per)
        win = rows // WROWS
        lrow = rows - win * WROWS
        b_loc = d_loc // 64
        cnt = np.zeros((nb, W), np.int64)
        np.add.at(cnt, (b_loc, win), 1)
        counts.append(cnt)
        core_edges.append((b_loc, win, lrow, d_loc - 64 * b_loc))
    K, directory, NW, stream_pos = _plan_chunks(cfg, counts)
    nch = len(directory)
    Kmax = int(K.max())
    SPOS = np.zeros((nb, W, Kmax), np.int64)
    for (b, w, k), v in stream_pos.items():
        SPOS[b, w, k] = v
    cfg.update(nch=nch, NW=tuple(NW),
               K=tuple(tuple(int(v) for v in row) for row in K))

    iota64 = np.broadcast_to(np.arange(64, dtype=np.float32), (128, 64)).copy()
    iotaG = np.broadcast_to(np.arange(g, dtype=np.float32), (128, g)).copy()
    ones_col = np.ones((128, 1), np.float32)
    I32rep = np.tile(np.eye(32, dtype=np.float32), (2, 1))
    IG = np.eye(g, dtype=np.float32)
    W2rep = np.concatenate([W2, W2], axis=0)
    bp1 = b1[np.arange(128) % hid].reshape(128, 1).astype(np.float32)
    bp2 = b2[np.arange(128) % hid].reshape(128, 1).astype(np.float32)
    bsrep = np.tile(bs.reshape(1, -1), (g, 1)).astype(np.float32)

    # window-stream column offsets
    CWOFF = np.concatenate([[0], np.cumsum(NW)]).astype(np.int64)

    in_maps = []
    for c in range(nc_):
        b_loc, win, lrow, dloc64 = core_edges[c]
        cnt = counts[c]
        # slot position of each edge: chunk (b,w,k) at stream col
        # CWOFF[w]+stream_pos[(b,w,k)], slot p in [0,128)
        # edges of (b,w) fill k=0.. sequentially
        idx_stream = np.zeros(nch * 128, np.int64)   # pad -> row 0
        dstl_stream = np.full(nch * 128, 64.0, np.float32)
        # order edges by (b, w) then position
        eorder = np.lexsort((win, b_loc))
        bo, wo, lo_, do_ = (b_loc[eorder], win[eorder], lrow[eorder],
                            dloc64[eorder])
        # running position within each (b, w) group
        grp = bo * W + wo
        gstart = np.zeros(nb * W, np.int64)
        np.add.at(gstart, grp, 1)
        goff = np.concatenate([[0], np.cumsum(gstart[:-1])])
        within = np.arange(len(bo)) - goff[grp]
        kk = within // 128
        p_in = within % 128
        scol = CWOFF[wo] + SPOS[bo, wo, kk]
        pos = scol * 128 + p_in
        idx_stream[pos] = lo_
        dstl_stream[pos] = do_.astype(np.float32)

        # idx16 packed per-window then concatenated: col range of window w
        # is [8*CWOFF[w], 8*CWOFF[w+1])
        idx16 = np.zeros((128, 8 * nch), np.int16)
        for w in range(W):
            seg = idx_stream[CWOFF[w] * 128: CWOFF[w + 1] * 128]
            F = len(seg) // 16
            a = seg.reshape(F, 16).T.astype(np.int16)
            idx16[:, 8 * CWOFF[w]: 8 * CWOFF[w + 1]] = np.tile(a, (8, 1))
        dstl_all = np.ascontiguousarray(
            dstl_stream.reshape(nch, 128).T)

        xs = np.zeros((npad, in_ch), np.float32)
        xs[:per] = x[c * per:(c + 1) * per]
        xT = np.ascontiguousarray(xs.T)

        degp = np.full(npad, 1e30, np.float32)
        degp[:per] = deg[c * per:(c + 1) * per]
        degP = np.empty((64, T // 2 * 128), np.float32)
        for t in range(T):
            blk = degp[t * 128:(t + 1) * 128][None, :].repeat(32, 0)
            degP[32 * (t % 2):32 * (t % 2) + 32,
                 128 * (t // 2):128 * (t // 2) + 128] = blk

        bb = np.full(npad, float(g), np.float32)
        bb[:per] = batch[c * per:(c + 1) * per].astype(np.float32)
        batch_cols = np.ascontiguousarray(bb.reshape(T, 128).T)

        in_maps.append({
            "xT": xT, "idx16": idx16, "dstl_all": dstl_all,
            "degP": degP, "batch_cols": batch_cols,
            "W1": W1, "W2rep": W2rep, "Ws": Ws,
            "bp1": bp1, "bp2": bp2, "bsrep": bsrep,
            "iota64": iota64, "iotaG": iotaG, "ones_col": ones_col,
            "I32rep": I32rep, "IG": IG,
        })

    return in_maps, cfg


# ======================================================================
# Device program
# ======================================================================

def build_nc(cfg):
    per, npad, T, nch = cfg["per"], cfg["npad"], cfg["T"], cfg["nch"]
    in_ch, hid, g, nc_ = cfg["in_ch"], cfg["hid"], cfg["g"], cfg["nc"]
    nb, W, WROWS = cfg["nb"], cfg["W"], cfg["WROWS"]
    K = cfg["K"]
    NW = cfg["NW"]
    assert hid == 32 and in_ch == 128 and T % 2 == 0
    PG = T // 2
    PK = PG * 128

    CWOFF = [0]
    for w in range(W):
        CWOFF.append(CWOFF[-1] + NW[w])

    # stream position per chunk
    stream_pos = {}
    pw = [0] * W
    for b in range(nb):
        for w in range(W):
            for k in range(K[b][w]):
                stream_pos[(b, w, k)] = pw[w]
                pw[w] += 1

    # batches of BP pairs; each batch -> per-window call segments
    batches = []
    for pg0 in range(0, PG, BP):
        pairs = list(range(pg0, min(pg0 + BP, PG)))
        bands = [b for p in pairs for b in range(4 * p, 4 * p + 4)]
        segs = []
        for w in range(W):
            seg = [(b, w, k) for b in bands for k in range(K[b][w])]
            for s0 in range(0, len(seg), MAX_SEG):
                sub = seg[s0:s0 + MAX_SEG]
                if sub:
                    segs.append((w, stream_pos[sub[0]], sub))
        batches.append((pairs, segs))

    nc = bacc.Bacc("TRN2", target_bir_lowering=False, debug=False,
                   num_devices=nc_)

    def din(name, shape, dt=F32):
        return nc.dram_tensor(name, shape, dt, kind="ExternalInput").ap()

    xT = din("xT", [in_ch, npad])
    idx16_i = din("idx16", [128, 8 * nch], I16)
    dstl_i = din("dstl_all", [128, nch])
    degP_i = din("degP", [64, PK])
    batch_i = din("batch_cols", [128, T])
    W1_i = din("W1", [in_ch, hid])
    W2rep_i = din("W2rep", [64, hid])
    Ws_i = din("Ws", [hid, hid])
    bp1_i = din("bp1", [128, 1])
    bp2_i = din("bp2", [128, 1])
    bsrep_i = din("bsrep", [g, hid])
    iota64_i = din("iota64", [128, 64])
    iotaG_i = din("iotaG", [128, g])
    ones_i = din("ones_col", [128, 1])
    I32rep_i = din("I32rep", [64, 32])
    IG_i = din("IG", [g, g])

    h_out = nc.dram_tensor("h_out", [npad, hid], F32,
                           kind="ExternalOutput").ap()
    summary_out = nc.dram_tensor("summary", [g, hid], F32,
                                 kind="ExternalOutput").ap()

    groups = [list(range(nc_))]

    with tile.TileContext(nc) as tc:
        with tc.tile_pool(name="const", bufs=1) as cp, \
             tc.tile_pool(name="persist", bufs=1) as pp, \
             tc.tile_pool(name="dram", bufs=1, space="DRAM") as dp:

            W1_t = cp.tile([in_ch, hid], F32)
            W2_t = cp.tile([64, hid], F32)
            Ws_t = cp.tile([hid, hid], F32)
            bp1_t = cp.tile([128, 1], F32)
            bp2_t = cp.tile([128, 1], F32)
            bsrep_t = cp.tile([g, hid], F32)
            iota64_t = cp.tile([128, 64], F32)
            iotaG_t = cp.tile([128, g], F32)
            ones_t = cp.tile([128, 1], F32)
            I32_t = cp.tile([64, 32], F32)
            IG_t = cp.tile([g, g], F32)
            batch_t = cp.tile([128, T], F32)
            dstl_t = pp.tile([128, nch], F32)
            dinvP_t = pp.tile([64, PK], F32)
            for dt_, sa in [(W1_t, W1_i), (W2_t, W2rep_i), (Ws_t, Ws_i),
                            (bp1_t, bp1_i), (bp2_t, bp2_i),
                            (bsrep_t, bsrep_i), (iota64_t, iota64_i),
                            (iotaG_t, iotaG_i), (ones_t, ones_i),
                            (I32_t, I32rep_i), (IG_t, IG_i),
                            (batch_t, batch_i), (dstl_t, dstl_i),
                            (dinvP_t, degP_i)]:
                nc.sync.dma_start(out=dt_[:], in_=sa[:])

            nc.scalar.activation(out=dinvP_t[:], in_=dinvP_t[:],
                                 func=mybir.ActivationFunctionType.Sqrt)
            nc.vector.reciprocal(out=dinvP_t[:], in_=dinvP_t[:])

            hs1T = pp.tile([64, PK], F32)
            h1pT = pp.tile([64, PK], F32)
            hs2T = pp.tile([64, PK], F32)

            bounce1 = dp.tile([npad, 64], F32)
            bounce2 = dp.tile([npad, 64], F32)
            table1 = dp.tile([nc_ * npad, 64], F32)
            table2 = dp.tile([nc_ * npad, 64], F32)
            pool_bounce = dp.tile([g, hid + 1], F32)
            pool_red = dp.tile([g, hid + 1], F32)

            # ---------------- P1: h1T = W1^T xT ; hs1T = dinv*h1T ---------
            with tc.tile_pool(name="xslab", bufs=3) as xp, \
                 tc.tile_pool(name="p1ps", bufs=2, space="PSUM") as p1p:
                for pg in range(PG):
                    xs = xp.tile([in_ch, 256], F32)
                    nc.sync.dma_start(out=xs[:],
                                      in_=xT[:, 256 * pg:256 * (pg + 1)])
                    hps = p1p.tile([64, 128], F32, space="PSUM")
                    for ti in range(2):
                        nc.tensor.matmul(
                            out=hps[32 * ti:32 * ti + 32, :],
                            lhsT=W1_t[:],
                            rhs=xs[:, 128 * ti:128 * (ti + 1)],
                            start=True, stop=True, skip_group_check=True)
                    nc.vector.tensor_mul(
                        out=hs1T[:, 128 * pg:128 * (pg + 1)],
                        in0=hps[:], in1=dinvP_t[:, 128 * pg:128 * (pg + 1)])

            # -------- un-transpose packed [64,PK] -> row tiles -> DRAM ----
            def untranspose_store(srcT, dram_ap, tag, pool_hook=None,
                                  wide=False):
                rw = 64 if wide else hid
                with tc.tile_pool(name=f"ut{tag}", bufs=2,
                                  space="PSUM") as utp, \
                     tc.tile_pool(name=f"ur{tag}", bufs=3) as urp:
                    for t in range(T):
                        o = 32 * (t % 2)
                        tp = utp.tile([128, hid], F32, space="PSUM")
                        nc.tensor.transpose(
                            out=tp[:],
                            in_=srcT[o:o + 32,
                                     128 * (t // 2):128 * (t // 2) + 128],
                            identity=I32_t[o:o + 32, :])
                        row = urp.tile([128, rw], F32, tag=f"rows{rw}")
                        if wide and t < 3:
                            nc.vector.memset(row[:, hid:rw], 0.0)
                        nc.scalar.copy(out=row[:, 0:hid], in_=tp[:])
                        nc.sync.dma_start(
                            out=dram_ap[128 * t:128 * (t + 1), 0:rw],
                            in_=row[:])
                        if pool_hook is not None:
                            pool_hook(t, row)

            untranspose_store(hs1T, bounce1[:], "a", wide=True)
            nc.gpsimd.collective_compute(
                "AllGather", mybir.AluOpType.bypass, replica_groups=groups,
                ins=[bounce1.opt()], outs=[table1.opt()])

            # ---------------- aggregation pass (shared for L1/L2) ---------
            def aggregate(table_tile, selfT, outT, bias_t, relu, tag):
                with tc.tile_pool(name=f"agg_idx{tag}", bufs=3) as ip, \
                     tc.tile_pool(name=f"agg_msg{tag}", bufs=3) as mp, \
                     tc.tile_pool(name=f"agg_s{tag}", bufs=3) as sp2, \
                     tc.tile_pool(name=f"agg_ps{tag}", bufs=BP + 1,
                                  space="PSUM") as ap2, \
                     tc.tile_pool(name=f"agg_ep{tag}", bufs=2) as ep:
                    for pairs, segs in batches:
                        pair_ps = {pg: ap2.tile([64, 128], F32, space="PSUM",
                                                name="aggps", tag="aggps")
                                   for pg in pairs}
                        remaining = {pg: sum(K[b][w] for b in
                                             range(4 * pg, 4 * pg + 4)
                                             for w in range(W))
                                     for pg in pairs}
                        t_remaining = {}
                        t_started = {}
                        for pg in pairs:
                            for tt in (2 * pg, 2 * pg + 1):
                                t_remaining[tt] = sum(
                                    K[b][w] for b in (2 * tt, 2 * tt + 1)
                                    for w in range(W))
                                t_started[tt] = False
                        for w, pw0, sub in segs:
                            ncw = len(sub)
                            iseg = ip.tile([128, 8 * ncw], I16, name="iseg",
                                           tag="iseg")
                            c0 = 8 * (CWOFF[w] + pw0)
                            nc.sync.dma_start(
                                out=iseg[:], in_=idx16_i[:, c0:c0 + 8 * ncw])
                            mseg = mp.tile([128, ncw * hid], F32,
                                           name="mseg", tag="mseg")
                            dma_gather_rows(
                                nc.gpsimd,
                                out_ap=mseg[:].rearrange(
                                    "p (c e) -> p c e", c=ncw),
                                in_ap=table_tile[
                                    w * WROWS:(w + 1) * WROWS, 0:hid],
                                idxs_ap=iseg[:],
                                num_idxs=ncw * 128,
                                elem_size=hid, elem_step=64)
                            sseg = sp2.tile([128, ncw * 64], F32,
                                            name="sseg", tag="sseg")
                            dl = dstl_t[:, CWOFF[w] + pw0:
                                        CWOFF[w] + pw0 + ncw]
                            in0 = bass.AP(dl.tensor, dl.offset,
                                          [dl.ap[0], [dl.ap[1][0], ncw],
                                           [0, 64]])
                            io = iota64_t[:]
                            in1 = bass.AP(io.tensor, io.offset,
                                          [io.ap[0], [0, ncw],
                                           [io.ap[1][0], 64]])
                            nc.vector.tensor_tensor(
                                out=sseg[:].rearrange(
                                    "p (c j) -> p c j", c=ncw),
                                in0=in0, in1=in1,
                                op=mybir.AluOpType.is_equal)
                            for j, (b, _, kchunk) in enumerate(sub):
                                t = b // 2
                                bit = b % 2
                                pg = t // 2
                                t_remaining[t] -= 1
                                nc.tensor.matmul(
                                    out=pair_ps[pg][
                                        32 * (t % 2):32 * (t % 2) + 32,
                                        64 * bit:64 * bit + 64],
                                    lhsT=mseg[:, hid * j:hid * (j + 1)],
                                    rhs=sseg[:, 64 * j:64 * (j + 1)],
                                    start=not t_started[t],
                                    stop=(t_remaining[t] == 0),
                                    skip_group_check=True)
                                t_started[t] = True
                                remaining[pg] -= 1
                                if remaining[pg] == 0:
                                    ps = pair_ps.pop(pg)
                                    sl = slice(128 * pg, 128 * (pg + 1))
                                    tmp = ep.tile([64, 128], F32,
                                                  name="etmp", tag="etmp")
                                    nc.vector.tensor_add(
                                        out=tmp[:], in0=ps[:],
                                        in1=selfT[:, sl])
                                    nc.vector.tensor_mul(
                                        out=tmp[:], in0=tmp[:],
                                        in1=dinvP_t[:, sl])
                                    if relu:
                                        nc.scalar.activation(
                                            out=outT[:, sl], in_=tmp[:],
                                            func=(mybir.
                                                  ActivationFunctionType.
                                                  Relu),
                                            bias=bias_t[0:64, :])
                                    else:
                                        nc.vector.tensor_scalar_add(
                                            out=outT[:, sl], in0=tmp[:],
                                            scalar1=bias_t[0:64, :])
                        assert not pair_ps

            # ---------------- P4: layer-1 aggregation ---------------------
            aggregate(table1, hs1T, h1pT, bp1_t, relu=True, tag="a")

            # ---------------- P5: h2T = W2^T h1pT ; hs2T ; exchange -------
            with tc.tile_pool(name="p5ps", bufs=2, space="PSUM") as p5p:
                for pg in range(PG):
                    hps = p5p.tile([64, 128], F32, space="PSUM")
                    for ti in range(2):
                        o = 32 * ti
                        nc.tensor.matmul(
                            out=hps[o:o + 32, :],
                            lhsT=W2_t[o:o + 32, :],
                            rhs=h1pT[o:o + 32, 128 * pg:128 * (pg + 1)],
                            start=True, stop=True, skip_group_check=True)
                    nc.vector.tensor_mul(
                        out=hs2T[:, 128 * pg:128 * (pg + 1)],
                        in0=hps[:], in1=dinvP_t[:, 128 * pg:128 * (pg + 1)])

            untranspose_store(hs2T, bounce2[:], "b", wide=True)
            nc.gpsimd.collective_compute(
                "AllGather", mybir.AluOpType.bypass, replica_groups=groups,
                ins=[bounce2.opt()], outs=[table2.opt()])

            # ---------------- P7: layer-2 aggregation (into h1pT buf) -----
            aggregate(table2, hs2T, h1pT, bp2_t, relu=False, tag="b")

            # ---------------- P8: un-transpose h -> h_out + pooling -------
            with tc.tile_pool(name="pool", bufs=2) as gp, \
                 tc.tile_pool(name="poolps", bufs=1, space="PSUM") as gpp:
                gsum_ps = gpp.tile([g, hid], F32, space="PSUM")
                cnt_ps = gpp.tile([g, 1], F32, space="PSUM")

                def pool_hook(t, row):
                    G_t = gp.tile([128, g], F32, name="G_t", tag="G")
                    nc.vector.tensor_scalar(
                        out=G_t[:], in0=iotaG_t[:],
                        scalar1=batch_t[:, t:t + 1], scalar2=None,
                        op0=mybir.AluOpType.is_equal)
                    nc.tensor.matmul(out=gsum_ps[:], lhsT=G_t[:],
                                     rhs=row[:],
                                     start=(t == 0), stop=(t == T - 1),
                                     skip_group_check=True)
                    nc.tensor.matmul(out=cnt_ps[:], lhsT=G_t[:],
                                     rhs=ones_t[:],
                                     start=(t == 0), stop=(t == T - 1),
                                     skip_group_check=True)

                untranspose_store(h1pT, h_out, "c", pool_hook=pool_hook)

                with tc.tile_pool(name="sumps", bufs=1, space="PSUM") as sps, \
                     tc.tile_pool(name="fin", bufs=1) as fp:
                    pool_sb = fp.tile([g, hid + 1], F32)
                    nc.vector.tensor_copy(out=pool_sb[:, 0:hid],
                                          in_=gsum_ps[:])
                    nc.vector.tensor_copy(out=pool_sb[:, hid:hid + 1],
                                          in_=cnt_ps[:])
                    nc.sync.dma_start(out=pool_bounce[:], in_=pool_sb[:])
                    nc.gpsimd.collective_compute(
                        "AllReduce", mybir.AluOpType.add,
                        replica_groups=groups,
                        ins=[pool_bounce.opt()], outs=[pool_red.opt()])
                    red_sb = fp.tile([g, hid + 1], F32)
                    nc.sync.dma_start(out=red_sb[:], in_=pool_red[:])
                    rec = fp.tile([g, 1], F32)
                    nc.vector.tensor_scalar_max(
                        out=rec[:], in0=red_sb[:, hid:hid + 1], scalar1=1.0)
                    nc.vector.reciprocal(out=rec[:], in_=rec[:])
                    gmean = fp.tile([g, hid], F32)
                    nc.vector.tensor_scalar_mul(
                        out=gmean[:], in0=red_sb[:, 0:hid], scalar1=rec[:])
                    gmt_ps = sps.tile([hid, g], F32, space="PSUM")
                    nc.tensor.transpose(out=gmt_ps[:], in_=gmean[:],
                                        identity=IG_t[:])
                    gmt = fp.tile([hid, g], F32)
                    nc.scalar.copy(out=gmt[:], in_=gmt_ps[:])
                    sum_ps = sps.tile([g, hid], F32, space="PSUM")
                    nc.tensor.matmul(out=sum_ps[:], lhsT=gmt[:], rhs=Ws_t[:],
                                     start=True, stop=True,
                                     skip_group_check=True)
                    sum_sb = fp.tile([g, hid], F32)
                    nc.vector.tensor_add(out=sum_sb[:], in0=sum_ps[:],
                                         in1=bsrep_t[:])
                    nc.sync.dma_start(out=summary_out[:], in_=sum_sb[:])

    nc.compile()
    return nc


# ======================================================================
# Entry point
# ======================================================================

_CACHE = {}


def _run(inputs, trace=False):
    in_maps, cfg = _prep(**inputs)
    key = tuple(sorted((k, v) for k, v in cfg.items()))
    if key not in _CACHE:
        _CACHE[key] = build_nc(cfg)
    nc = _CACHE[key]
    res = bass_utils.run_bass_kernel_spmd(
        nc, in_maps, core_ids=list(range(cfg["nc"])), trace=trace)
    per = cfg["per"]
    h = np.concatenate([res.results[c]["h_out"][:per]
                        for c in range(cfg["nc"])], axis=0)
    summary = res.results[0]["summary"]
    return (summary, h), res


def kernel(**inputs):
    out, _ = _run(inputs, trace=False)
    return out
